# revision 1
# baseline (speedup 1.0000x reference)
"""Trainium2 Bass kernel for nn_Activation2d (anti-aliased activation):
   y = downsample2d(leaky_relu(upsample2d(x)))  on x [8, 64, 256, 256] fp32.

Algorithm: both resamplers are separable 1D kaiser-sinc filters, expressed as
banded matrices baked with edge-replication clamping:
  U = A X          (up along H;   A [512,256], includes ratio factor 2)
  V = U A^T        (up along W)
  L = lrelu(V)
  D = L B^T        (down along W; B [256,512])
  Y = B D          (down along H)

On the PE this is 4 matmul passes per image, alternating transposing /
standard forms so each pass's contraction lands on the partition axis:
  P1 (contract h):  out1 = X^T-conv     lhsT = X        rhs = A^T   -> [w, n]
  P2 (contract w):  out2                lhsT = A^T      rhs = out1  -> [w', n]
  lrelu fused into PSUM->SBUF copy on ACT (Prelu, alpha=0.2)
  P3 (contract w'): out3                lhsT = L        rhs = B^T   -> [n, m]
  P4 (contract n):  out4                lhsT = B^T      rhs = out3  -> [h'', m]

dtypes: P1/P2/P4 in float32r (fp32 with ~12-bit mantissa, full PE speed at
free-dim >= 256), P3 in fp16 (band-limited streams < 256 would put fp32r at
4x cost). Accumulation is always fp32 in PSUM.

Sharding: pure data parallel over batch — core b computes x[b] [64,256,256].
"""
import math
from contextlib import ExitStack

import numpy as np
import ml_dtypes

import concourse.bass as bass
import concourse.bacc as bacc
import concourse.tile as tile
import concourse.mybir as mybir
from concourse.bass_utils import run_bass_kernel_spmd

RATIO = 2
KSIZE = 12
SLOPE = 0.2
H = W = 256
NCORES = 8

F32R = mybir.dt.float32r
F16 = mybir.dt.float16
F32 = mybir.dt.float32


# ----------------------------------------------------------------------------
# filter construction (mirrors the reference's kaiser_sinc_filter1d)
# ----------------------------------------------------------------------------
def _kaiser_sinc_filter1d(cutoff, half_width, kernel_size):
    half_size = kernel_size // 2
    delta_f = 4.0 * half_width
    A = 2.285 * (half_size - 1) * math.pi * delta_f + 7.95
    if A > 50.0:
        beta = 0.1102 * (A - 8.7)
    elif A >= 21.0:
        beta = 0.5842 * (A - 21.0) ** 0.4 + 0.07886 * (A - 21.0)
    else:
        beta = 0.0
    window = np.kaiser(kernel_size, beta)
    if kernel_size % 2 == 0:
        time = np.arange(-half_size, half_size) + 0.5
    else:
        time = np.arange(kernel_size) - half_size
    filt = 2.0 * cutoff * window * np.sinc(2.0 * cutoff * time)
    filt = filt / filt.sum()
    return filt.astype(np.float32)


def build_A(n_in=H):
    f = _kaiser_sinc_filter1d(0.5 / RATIO, 0.6 / RATIO, KSIZE).astype(np.float64)
    A = np.zeros((2 * n_in, n_in), np.float64)
    for t in range(n_in):
        for j in range(6):
            A[2 * t, np.clip(t + j - 3, 0, n_in - 1)] += 2.0 * f[2 * j]
            A[2 * t + 1, np.clip(t + j - 2, 0, n_in - 1)] += 2.0 * f[2 * j + 1]
    return A.astype(np.float32)


def build_B(n_out=H):
    f = _kaiser_sinc_filter1d(0.5 / RATIO, 0.6 / RATIO, KSIZE).astype(np.float64)
    B = np.zeros((n_out, 2 * n_out), np.float64)
    for m in range(n_out):
        for k in range(KSIZE):
            B[m, np.clip(2 * m + k - 5, 0, 2 * n_out - 1)] += f[k]
    return B.astype(np.float32)


def _nz_cols(mat, even=False):
    """[lo, hi) column range containing all nonzeros of mat.
    even=True rounds outward to even offsets/counts (fp32r matmul ISA rule:
    src/dst free dims must be even-count, 8B-aligned)."""
    nz = np.nonzero(np.any(mat != 0.0, axis=0))[0]
    lo, hi = int(nz[0]), int(nz[-1]) + 1
    if even:
        lo -= lo % 2
        hi += hi % 2
    return lo, hi


# ----------------------------------------------------------------------------
# bass program
# ----------------------------------------------------------------------------
DT = {"f32r": mybir.dt.float32r, "f16": mybir.dt.float16, "bf16": mybir.dt.bfloat16}
NPDT = {"f16": np.float16, "bf16": ml_dtypes.bfloat16}


def build_nc(n_img=64, repeats=1, in_batch=4, cast_engine="vector",
             dt_p1="f32r", dt_p2="f32r", dt_p4="f32r",
             in_parity=False, out_poly=False, merge_copies=False,
             psum_bufs=None, dma_cast=False, skew=True, shared_psum=False,
             pair_p2=False, p3_korder=True,
             sbuf_bufs=(2, 4, 10, 4, 3)):
    A = build_A()          # [512, 256]
    B = build_B()          # [256, 512]
    AT = A.T.copy()        # [256, 512] rows h, cols n
    BT = B.T.copy()        # [512, 256] rows n/w', cols m/h''

    # ---- P1 constants: rows of A^T grouped by h-layout of the X tiles ----
    if in_parity:
        # X tile partition p holds rows h = 2p (block b=0) and 2p+1 (b=1):
        # contiguous 2KB DMA lines. A^T rows permuted to match.
        AT1 = [AT[b::2, :].copy() for b in range(2)]
        p1_win = [(0, 512), (0, 512)]
    else:
        AT1 = [AT[128 * b:128 * (b + 1), :].copy() for b in range(2)]
        p1_win = [_nz_cols(m, even=True) for m in AT1]
        assert p1_win[0][0] == 0 and p1_win[1][1] == 512

    # P2: lhsT = A^T[w-blk b, w'-tile q] (natural w layout; U tiles hold
    # w = p + 128b). nonzero (q, b) pairs:
    AT2 = [AT[128 * b:128 * (b + 1), :].copy() for b in range(2)]
    p2_blocks = [
        [b for b in range(2)
         if np.any(AT2[b][:, 128 * q:128 * (q + 1)] != 0.0)]
        for q in range(4)
    ]

    # P3: rhs m-windows per w'-block (cols of B^T rows blk)
    p3_win = [_nz_cols(BT[128 * k:128 * (k + 1)]) for k in range(4)]
    cov = np.zeros(256, bool)
    for lo, hi in p3_win:
        cov[lo:hi] = True
    assert cov.all()

    # ---- P4 constants: cols of B^T in output layout order ----------------
    if out_poly:
        # out4[p, 256t + w] = y[2p + t, w]: contiguous 2KB output DMA lines.
        # lhsT for block t = B^T[:, t::2]; band spans all n -> all 4 k-blocks.
        BT4 = np.concatenate([BT[:, 0::2], BT[:, 1::2]], axis=1)  # [512, 256]
    else:
        BT4 = BT
    p4_blocks = [
        [k for k in range(4)
         if np.any(BT4[128 * k:128 * (k + 1), 128 * t:128 * (t + 1)] != 0.0)]
        for t in range(2)
    ]

    nc = bacc.Bacc("TRN2", target_bir_lowering=False, debug=False,
                   num_devices=NCORES)
    x_ap = nc.dram_tensor("x", [n_img, H, W], F32, kind="ExternalInput").ap()
    y_ap = nc.dram_tensor("y", [n_img, H, W], F32, kind="ExternalOutput").ap()

    const_drams = {}

    def const_dram(mat, name, dt_name):
        key = (name, dt_name)
        if key not in const_drams:
            if dt_name == "f32r":
                const_drams[key] = nc.inline_tensor(
                    np.ascontiguousarray(mat).astype(np.float32), name=f"{name}_f32")
            else:
                const_drams[key] = nc.inline_tensor(
                    np.ascontiguousarray(mat).astype(NPDT[dt_name]),
                    name=f"{name}_{dt_name}")
        return const_drams[key]

    with tile.TileContext(nc) as tc, ExitStack() as ctx:
        cpool = ctx.enter_context(tc.tile_pool(name="consts", bufs=1))
        xpool = ctx.enter_context(tc.tile_pool(name="xin", bufs=sbuf_bufs[0]))
        upool = ctx.enter_context(tc.tile_pool(name="u", bufs=sbuf_bufs[1]))
        lpool = ctx.enter_context(tc.tile_pool(name="l", bufs=sbuf_bufs[2]))
        dpool = ctx.enter_context(tc.tile_pool(name="d", bufs=sbuf_bufs[3]))
        opool = ctx.enter_context(tc.tile_pool(name="o", bufs=sbuf_bufs[4]))
        if shared_psum:
            # P1/P3/P4 tiles all [128,512]=1 bank: share one tag/pool so banks
            # recycle across stages; P2 gets a full 4-bank double of its own.
            pb = psum_bufs or (4, 4)
            ppA = ctx.enter_context(tc.tile_pool(name="ppA", bufs=pb[0], space="PSUM"))
            pp2 = ctx.enter_context(tc.tile_pool(name="pp2", bufs=pb[1], space="PSUM"))
            pp1 = pp3 = pp4 = ppA
            psum_tag = lambda which: "psA"
        elif merge_copies:
            pb = psum_bufs or (1, 3, 1, 1)
            pp1 = ctx.enter_context(tc.tile_pool(name="pp1", bufs=pb[0], space="PSUM"))
            pp2 = ctx.enter_context(tc.tile_pool(name="pp2", bufs=pb[1], space="PSUM"))
            pp3 = ctx.enter_context(tc.tile_pool(name="pp3", bufs=pb[2], space="PSUM"))
            pp4 = ctx.enter_context(tc.tile_pool(name="pp4", bufs=pb[3], space="PSUM"))
            psum_tag = lambda which: which
        else:
            pb = psum_bufs or (2, 3, 2, 1)
            pp1 = ctx.enter_context(tc.tile_pool(name="pp1", bufs=pb[0], space="PSUM"))
            pp2 = ctx.enter_context(tc.tile_pool(name="pp2", bufs=pb[1], space="PSUM"))
            pp3 = ctx.enter_context(tc.tile_pool(name="pp3", bufs=pb[2], space="PSUM"))
            pp4 = ctx.enter_context(tc.tile_pool(name="pp4", bufs=pb[3], space="PSUM"))
            psum_tag = lambda which: which

        # ---- constants: 16-bit shipped pre-rounded; f32r rounded on-chip
        _const_tiles = {}

        def const_tiles(mat, name, dt_name, tile_cols):
            key = (name, dt_name)
            if key in _const_tiles:
                return _const_tiles[key]
            dram = const_dram(mat, name, dt_name)
            tiles = []
            for b in range(mat.shape[0] // 128):
                sl = dram.ap()[128 * b:128 * (b + 1), :]
                if dt_name == "f32r":
                    stg = cpool.tile([128, tile_cols], F32, tag=f"{name}_stg{b}")
                    nc.sync.dma_start(stg[:], sl)
                    t = cpool.tile([128, tile_cols], F32R, tag=f"{name}_{dt_name}{b}")
                    nc.vector.tensor_copy(t[:], stg[:])
                else:
                    t = cpool.tile([128, tile_cols], DT[dt_name], tag=f"{name}_{dt_name}{b}")
                    nc.sync.dma_start(t[:], sl)
                tiles.append(t)
            _const_tiles[key] = tiles
            return tiles

        AT_p1 = const_tiles(np.concatenate(AT1, 0), "at1", dt_p1, 512)  # P1 rhs
        AT_p2 = const_tiles(np.concatenate(AT2, 0), "at2", dt_p2, 512)  # P2 lhsT
        BT16 = const_tiles(BT, "bt3", "f16", 256)                       # P3 rhs
        BT_p4 = const_tiles(BT4, "bt4", dt_p4, 256)                     # P4 lhsT

        # ---- per-image pipeline ----------------------------------------
        xr_tiles = {}  # c -> (tile, col offset)
        state = {}     # c -> dict with U, L, D aps
        img_seq = [i for _ in range(repeats) for i in range(n_img)]

        def stage1(idx, c):
            # -- input: HWDGE fp32 load (contiguous lines), engine cast --
            if idx % in_batch == 0:
                nb = min(in_batch, len(img_seq) - idx, n_img - c)
                xf = xpool.tile([128, nb * 512], F32, tag="xf")
                if in_parity:
                    src = x_ap[c:c + nb].rearrange("c (p b) w -> p c b w", p=128)
                else:
                    src = x_ap[c:c + nb].rearrange("c (b p) w -> p c b w", p=128)
                nc.sync.dma_start(
                    xf[:].rearrange("p (c b w) -> p c b w", c=nb, b=2), src)
                xt = xpool.tile([128, nb * 512], DT[dt_p1], tag="xr")
                if cast_engine == "swdge":
                    nc.gpsimd.dma_start(xt[:], xf[:])
                elif cast_engine == "pool":
                    nc.gpsimd.tensor_copy(xt[:], xf[:])
                else:
                    getattr(nc, cast_engine).tensor_copy(xt[:], xf[:])
                for i in range(nb):
                    xr_tiles[c + i] = (xt, 512 * i)
            xt, off = xr_tiles[c]
            Xr = [xt[:, off + 256 * b: off + 256 * (b + 1)] for b in range(2)]

            # -- P1: out1[w-blk] = sum_h X[h, w-blk] A^T[h, n] ------------
            if merge_copies:
                ps1 = pp1.tile([128, 1024], F32, tag="ps1")
                for b_out in range(2):
                    for i, b in enumerate(range(2)):
                        lo, hi = p1_win[b]
                        nc.tensor.matmul(
                            ps1[:, 512 * b_out + lo:512 * b_out + hi],
                            Xr[b][:, 128 * b_out:128 * (b_out + 1)],
                            AT_p1[b][:, lo:hi],
                            start=(i == 0), stop=(i == 1),
                        )
                u = upool.tile([128, 1024], DT[dt_p2], tag="u")
                nc.vector.tensor_copy(u[:], ps1[:])
                U = [u[:, 512 * b:512 * (b + 1)] for b in range(2)]
            else:
                U = []
                for b_out in range(2):
                    ps = pp1.tile([128, 512], F32, tag=psum_tag("ps1"))
                    for i, b in enumerate(range(2)):
                        lo, hi = p1_win[b]
                        nc.tensor.matmul(
                            ps[:, lo:hi],
                            Xr[b][:, 128 * b_out:128 * (b_out + 1)],
                            AT_p1[b][:, lo:hi],
                            start=(i == 0), stop=(i == 1),
                        )
                    uu = upool.tile([128, 512], DT[dt_p2], tag="u")
                    nc.vector.tensor_copy(uu[:], ps[:])
                    U.append(uu[:])
            state[c] = {"U": U}

        def stage2(idx, c):
            # -- P2: out2[w'-tile q] = sum_w A^T[w, w'-q] out1[w, n] ------
            U = state[c]["U"]
            L = []
            if pair_p2:
                for pair in range(2):
                    ps = pp2.tile([128, 1024], F32, tag="ps2")
                    for j in range(2):
                        q = 2 * pair + j
                        blocks = p2_blocks[q]
                        for i, b in enumerate(blocks):
                            nc.tensor.matmul(
                                ps[:, 512 * j:512 * (j + 1)],
                                AT_p2[b][:, 128 * q:128 * (q + 1)],
                                U[b],
                                start=(i == 0), stop=(i == len(blocks) - 1),
                            )
                    l = lpool.tile([128, 1024], F16, tag="l")
                    nc.scalar.activation(l[:], ps[:],
                                         mybir.ActivationFunctionType.Prelu,
                                         alpha=SLOPE)
                    L.append(l[:, 0:512])
                    L.append(l[:, 512:1024])
            else:
                for q in range(4):
                    ps = pp2.tile([128, 512], F32, tag="ps2")
                    blocks = p2_blocks[q]
                    for i, b in enumerate(blocks):
                        nc.tensor.matmul(
                            ps[:],
                            AT_p2[b][:, 128 * q:128 * (q + 1)],
                            U[b],
                            start=(i == 0), stop=(i == len(blocks) - 1),
                        )
                    l = lpool.tile([128, 512], F16, tag="l")
                    nc.scalar.activation(l[:], ps[:],
                                         mybir.ActivationFunctionType.Prelu,
                                         alpha=SLOPE)
                    L.append(l[:])
            state[c]["L"] = L

        def stage3(idx, c):
            # -- P3: out3[n-blk nb] in psum cols 256*nb -------------------
            L = state[c]["L"]
            if merge_copies:
                ps3 = pp3.tile([128, 1024], F32, tag="ps3")
                seen_banks = set()
                for k in range(4):          # k outer: 4 matmuls per lrelu(L[k])
                    lo, hi = p3_win[k]
                    for nb_ in range(4):
                        bank = nb_ // 2
                        nc.tensor.matmul(
                            ps3[:, 256 * nb_ + lo:256 * nb_ + hi],
                            L[k][:, 128 * nb_:128 * (nb_ + 1)],
                            BT16[k][:, lo:hi],
                            start=(bank not in seen_banks),
                            stop=(k == 3 and nb_ == 3),
                            skip_group_check=True,
                        )
                        seen_banks.add(bank)
                d = dpool.tile([128, 1024], DT[dt_p4], tag="d")
                nc.vector.tensor_copy(d[:], ps3[:])
                D = [d[:, 256 * k:256 * (k + 1)] for k in range(4)]
            else:
                D = []
                tiles3 = [pp3.tile([128, 512], F32, tag=psum_tag("ps3"), name="ps3a"),
                          pp3.tile([128, 512], F32, tag=psum_tag("ps3"), name="ps3b")]
                seen_banks = set()
                for k in range(4):          # k outer across both psum tiles
                    lo, hi = p3_win[k]
                    for nb_ in range(4):
                        ps = tiles3[nb_ // 2]
                        j = nb_ % 2
                        nc.tensor.matmul(
                            ps[:, 256 * j + lo:256 * j + hi],
                            L[k][:, 128 * nb_:128 * (nb_ + 1)],
                            BT16[k][:, lo:hi],
                            start=(nb_ // 2 not in seen_banks),
                            stop=(k == 3 and nb_ == 3),
                            skip_group_check=True,
                        )
                        seen_banks.add(nb_ // 2)
                for g in range(2):
                    dd = dpool.tile([128, 512], DT[dt_p4], tag="d")
                    nc.vector.tensor_copy(dd[:], tiles3[g][:])
                    D.append(dd[:, 0:256])
                    D.append(dd[:, 256:512])
            state[c]["D"] = D

        def stage4(idx, c):
            # -- P4: out4 [128, 512]: block t in cols 256t ----------------
            D = state[c]["D"]
            ps4 = pp4.tile([128, 512], F32, tag=psum_tag("ps4"))
            first = True
            for t in range(2):
                blocks = p4_blocks[t]
                for i, k in enumerate(blocks):
                    nc.tensor.matmul(
                        ps4[:, 256 * t:256 * (t + 1)],
                        BT_p4[k][:, 128 * t:128 * (t + 1)],
                        D[k],
                        start=first,
                        stop=(t == 1 and i == len(blocks) - 1),
                    )
                    first = False
            o = opool.tile([128, 512], F32, tag="o")
            nc.scalar.copy(o[:], ps4[:])
            if out_poly:
                nc.sync.dma_start(
                    y_ap[c].rearrange("(p t) w -> p (t w)", t=2), o[:])
            else:
                nc.sync.dma_start(
                    y_ap[c].rearrange("(t p) w -> p t w", p=128),
                    o[:].rearrange("p (t w) -> p t w", t=2))
            del state[c]

        # software-pipelined emission: step i emits S1(i), S3(i-1), S2(i),
        # S4(i-1) so PE always has same-FIFO filler work while copies drain.
        if skew:
            n = len(img_seq)
            for i in range(n + 1):
                if i < n:
                    stage1(i, img_seq[i])
                if i >= 1:
                    stage3(i - 1, img_seq[i - 1])
                if i < n:
                    stage2(i, img_seq[i])
                if i >= 1:
                    stage4(i - 1, img_seq[i - 1])
        else:
            for idx, c in enumerate(img_seq):
                stage1(idx, c)
                stage2(idx, c)
                stage3(idx, c)
                stage4(idx, c)

    nc.compile()
    return nc


_NC_CACHE = {}

# tuned configuration used by kernel()
BEST_CFG = dict(in_batch=4, dt_p1="f32r", dt_p2="f32r", dt_p4="f32r",
                cast_engine="pool")


def _get_nc(n_img, **overrides):
    cfg = dict(BEST_CFG, **overrides)
    key = (n_img, tuple(sorted((k, str(v)) for k, v in cfg.items())))
    if key not in _NC_CACHE:
        _NC_CACHE[key] = build_nc(n_img, **cfg)
    return _NC_CACHE[key]


def kernel(x: np.ndarray) -> np.ndarray:
    """x: [8, 64, 256, 256] fp32 -> y same shape."""
    x = np.asarray(x, dtype=np.float32)
    assert x.shape == (NCORES, 64, H, W), x.shape
    nc = _get_nc(64)
    in_maps = [{"x": x[b]} for b in range(NCORES)]
    res = run_bass_kernel_spmd(nc, in_maps, core_ids=list(range(NCORES)))
    return np.stack([res.results[b]["y"] for b in range(NCORES)], axis=0)

